# revision 8
# baseline (speedup 1.0000x reference)
"""MoE FFN (8 experts, top-2, shared expert) on 8 trn2 NeuronCores.

Strategy (expert parallel, per sharding hint):
  - Host computes router logits + top-2 + softmax weights (0.04% of FLOPs),
    then dispatches: core e receives the tokens routed to expert e (padded to
    a fixed capacity) plus a 1/8 data-parallel slice of all tokens for the
    shared expert.
  - Each core runs a fused FFN kernel: h = silu(x@Wg) * (x@Wu); out = scale
    * (h @ Wd), where scale is the combine weight (expert segment) or 1.0
    (shared segment). All matmuls use fp16 operands (PE runs fp16 at full
    rate; fp32 accumulate in PSUM), weights stay SBUF-resident per segment.
  - Host scatter-adds the per-expert outputs and the shared outputs into the
    full [B,T,D] result.
"""

import os
import numpy as np

import concourse.bass as bass  # noqa: F401  (bass types used via bacc/tile)
import concourse.mybir as mybir
import concourse.tile as tile
from concourse import bacc
from concourse.bass_utils import run_bass_kernel_spmd

NCORES = 8
N_EXPERTS = 8
TOP_K = 2
D_MODEL = 1024
D_FF = 2048
B, T = 4, 2048
N_TOKENS = B * T          # 8192
NSHARED = N_TOKENS // NCORES  # 1024 shared-expert tokens per core
GROUP = 512               # token group processed per inner iteration

_nc_cache: dict = {}
last_results = None  # BassKernelResults of the most recent run (for test.py)


def _build_moe_nc(ttot: int, cap: int, d: int = D_MODEL, f: int = D_FF,
                  nshared: int = NSHARED, group: int = GROUP):
    """Build the single-core Bass program (SPMD across 8 cores).

    ttot = cap + nshared tokens per core. Expert segment [0, cap) uses
    wg/wu/wd with per-token combine scale; shared segment [cap, ttot) uses
    sg/su/sd with scale 1.0 (host bakes ones into the scale vector).
    """
    f16 = mybir.dt.float16
    f32 = mybir.dt.float32
    kd = d // 128    # k-tiles over d_model
    kf = f // 128    # k-tiles over d_ff
    nft = f // 128   # output f-tiles of phase A
    nch = d // 512   # 512-wide output column chunks of phase B

    nc = bacc.Bacc("TRN2", target_bir_lowering=False, debug=False,
                   num_devices=NCORES)
    xT = nc.dram_tensor("xT", [d, ttot], f16, kind="ExternalInput").ap()
    wg = nc.dram_tensor("wg", [d, f], f16, kind="ExternalInput").ap()
    wu = nc.dram_tensor("wu", [d, f], f16, kind="ExternalInput").ap()
    wd = nc.dram_tensor("wd", [f, d], f16, kind="ExternalInput").ap()
    sg = nc.dram_tensor("sg", [d, f], f16, kind="ExternalInput").ap()
    su = nc.dram_tensor("su", [d, f], f16, kind="ExternalInput").ap()
    sd = nc.dram_tensor("sd", [f, d], f16, kind="ExternalInput").ap()
    sc = nc.dram_tensor("scale", [ttot, 1], f32, kind="ExternalInput").ap()
    out = nc.dram_tensor("out", [ttot, d], f32, kind="ExternalOutput").ap()

    with tile.TileContext(nc) as tc:
        with (
            tc.tile_pool(name="wpool", bufs=1) as wpool,
            tc.tile_pool(name="xpool", bufs=3) as xpool,
            tc.tile_pool(name="hpool", bufs=3) as hpool,
            tc.tile_pool(name="spool", bufs=4) as spool,
            tc.tile_pool(name="opool", bufs=4) as opool,
            tc.tile_pool(name="cpool", bufs=1) as cpool,
            tc.tile_pool(name="psg", bufs=2, space="PSUM") as psg,
            tc.tile_pool(name="psu", bufs=2, space="PSUM") as psu,
            tc.tile_pool(name="pso", bufs=3, space="PSUM") as pso,
        ):
            xT_r = xT.rearrange("(k p) t -> p k t", p=128)

            def load_x(t0, g0, gl):
                xg = xpool.tile([128, kd, group], f16, name="xg")
                nc.sync.dma_start(out=xg[:, :, :gl],
                                  in_=xT_r[:, :, t0 + g0:t0 + g0 + gl])
                return xg

            def phase_a(seg, xg, g0, gl):
                """gate/up matmuls + silu*up for one token group."""
                wg_sb, wu_sb, _ = seg
                hgrp = hpool.tile([128, kf, group], f16, name="h")
                for ft in range(nft):
                    pg = psg.tile([128, group], f32, name="pg")
                    pu = psu.tile([128, group], f32, name="pu")
                    fsl = slice(ft * 128, (ft + 1) * 128)
                    for k in range(kd):
                        nc.tensor.matmul(pg[:, :gl], lhsT=wg_sb[:, k, fsl],
                                         rhs=xg[:, k, :gl],
                                         start=(k == 0), stop=(k == kd - 1))
                    for k in range(kd):
                        nc.tensor.matmul(pu[:, :gl], lhsT=wu_sb[:, k, fsl],
                                         rhs=xg[:, k, :gl],
                                         start=(k == 0), stop=(k == kd - 1))
                    sil = spool.tile([128, group], f32, name="sil")
                    nc.scalar.activation(
                        out=sil[:, :gl], in_=pg[:, :gl],
                        func=mybir.ActivationFunctionType.Silu)
                    nc.vector.tensor_mul(hgrp[:, ft, :gl], sil[:, :gl],
                                         pu[:, :gl])
                return hgrp

            def phase_b(seg, sc_sb, hgrp, t0, g0, gl):
                """down matmul + combine-weight scale + store."""
                _, _, wd_sb = seg
                for ts in range(gl // 128):
                    tok0 = t0 + g0 + ts * 128
                    tsl = slice(ts * 128, (ts + 1) * 128)
                    for ch in range(nch):
                        csl = slice(ch * 512, (ch + 1) * 512)
                        po = pso.tile([128, 512], f32, name="po")
                        for k in range(kf):
                            nc.tensor.matmul(po, lhsT=hgrp[:, k, tsl],
                                             rhs=wd_sb[:, k, csl],
                                             start=(k == 0), stop=(k == kf - 1))
                        ot = opool.tile([128, 512], f32, name="ot")
                        nc.scalar.mul(ot, po,
                                      sc_sb[:, tok0 // 128:tok0 // 128 + 1])
                        nc.sync.dma_start(out=out[tok0:tok0 + 128, csl],
                                          in_=ot)

            def load_weights(WG, WU, WD):
                wg_sb = wpool.tile([128, kd, f], f16, name="wg")
                wu_sb = wpool.tile([128, kd, f], f16, name="wu")
                wd_sb = wpool.tile([128, kf, d], f16, name="wd")
                WG_r = WG.rearrange("(k p) f -> p k f", p=128)
                WU_r = WU.rearrange("(k p) f -> p k f", p=128)
                WD_r = WD.rearrange("(k p) c -> p k c", p=128)
                # f-chunk interleaved loads in first-use order: the ft loop
                # consumes wg/wu low-f first, so chunk 0 landing unblocks the
                # first matmul chains while the rest streams in behind; wd is
                # only needed a full token-group later.
                fc = 512
                for c0 in range(0, f, fc):
                    nc.sync.dma_start(out=wg_sb[:, :, c0:c0 + fc],
                                      in_=WG_r[:, :, c0:c0 + fc])
                    nc.sync.dma_start(out=wu_sb[:, :, c0:c0 + fc],
                                      in_=WU_r[:, :, c0:c0 + fc])
                for k in range(kf):
                    nc.sync.dma_start(out=wd_sb[:, k, :], in_=WD_r[:, k, :])
                return wg_sb, wu_sb, wd_sb

            sc_sb = cpool.tile([128, ttot // 128], f32, name="sc")
            nc.sync.dma_start(out=sc_sb,
                              in_=sc.rearrange("(g p) o -> p (g o)", p=128))

            # software pipeline: phase B runs one token-group behind phase A
            # so the PE never waits on the silu/mul tail of the current group.
            # Shared segment first: the expert segment's short tail group
            # (cap % group) then ends the pipeline, minimizing the exposed
            # final phase B.
            work = []  # (t0, g0, gl, seg_idx)
            seg_descs = [(cap, nshared, sg, su, sd), (0, cap, wg, wu, wd)]
            for si, (t0, seglen, *_) in enumerate(seg_descs):
                for g0 in range(0, seglen, group):
                    work.append((t0, g0, min(group, seglen - g0), si))

            segs = {}
            pending = None  # (seg, hgrp, t0, g0, gl)
            for t0, g0, gl, si in work:
                if si not in segs:
                    # flush the pipeline before reusing the weight slots, so
                    # the new segment's weight DMAs overlap the flushed
                    # phase B instead of stalling queued PE work behind them
                    if pending is not None:
                        phase_b(*pending)
                        pending = None
                    xg = load_x(t0, g0, gl)  # x before the 12MB of weights
                    segs[si] = load_weights(*seg_descs[si][2:])
                else:
                    xg = load_x(t0, g0, gl)
                hgrp = phase_a(segs[si], xg, g0, gl)
                if pending is not None:
                    phase_b(*pending)
                pending = (segs[si], sc_sb, hgrp, t0, g0, gl)
            phase_b(*pending)
    nc.compile()
    return nc


def _route(xf: np.ndarray, router_w: np.ndarray):
    """Host router: top-2 experts + softmax combine weights per token.

    Mirrors the reference's jax fp32 arithmetic (matmul + lax.top_k +
    softmax) on the CPU backend so borderline top-2/top-3 ties resolve the
    same way the reference resolves them; falls back to numpy if jax is
    unavailable.
    """
    try:
        import jax
        import jax.numpy as jnp
        cpu = jax.local_devices(backend="cpu")[0]
        with jax.default_device(cpu):
            logits = jnp.matmul(jnp.asarray(xf), jnp.asarray(router_w).T)
            top_vals, top_idx = jax.lax.top_k(logits, TOP_K)
            top_w = jax.nn.softmax(top_vals, axis=-1)
        top_idx = np.asarray(top_idx)
        top_w = np.asarray(top_w, dtype=np.float32)
        return top_idx[:, 0], top_idx[:, 1], top_w[:, 0], top_w[:, 1]
    except Exception:
        n = xf.shape[0]
        logits = xf @ router_w.T.astype(np.float32)
        a = np.arange(n)
        top1 = np.argmax(logits, axis=1)
        v1 = logits[a, top1]
        l2 = logits.copy()
        l2[a, top1] = -np.inf
        top2 = np.argmax(l2, axis=1)
        v2 = l2[a, top2]
        e2 = np.exp(v2 - v1)
        w1 = (1.0 / (1.0 + e2)).astype(np.float32)
        w2 = (1.0 - w1).astype(np.float32)
        return top1, top2, w1, w2


def kernel(x, router_w, w_gate, w_up, w_down, sg, su, sd):
    global last_results
    trace = bool(int(os.environ.get("KERNEL_TRACE", "0")))

    xf = np.ascontiguousarray(np.asarray(x, dtype=np.float32).reshape(-1, D_MODEL))
    top1, top2, w1, w2 = _route(xf, np.asarray(router_w, dtype=np.float32))

    idx_list, wts_list = [], []
    for e in range(N_EXPERTS):
        i1 = np.nonzero(top1 == e)[0]
        i2 = np.nonzero(top2 == e)[0]
        idx_list.append(np.concatenate([i1, i2]))
        wts_list.append(np.concatenate([w1[i1], w2[i2]]).astype(np.float32))
    max_cnt = max(len(ix) for ix in idx_list)
    cap = max(128, -(-max_cnt // 128) * 128)
    ttot = cap + NSHARED

    key = (ttot, cap)
    if key not in _nc_cache:
        _nc_cache[key] = _build_moe_nc(ttot, cap)
    nc = _nc_cache[key]

    xT16 = np.ascontiguousarray(xf.T).astype(np.float16)  # [D, N]
    sg16, su16, sd16 = (np.asarray(a).astype(np.float16) for a in (sg, su, sd))
    in_maps = []
    for e in range(N_EXPERTS):
        ix, wt = idx_list[e], wts_list[e]
        cnt = len(ix)
        ix_pad = np.concatenate([ix, np.zeros(cap - cnt, dtype=ix.dtype)])
        xin = np.empty((D_MODEL, ttot), dtype=np.float16)
        xin[:, :cap] = xT16[:, ix_pad]
        xin[:, cap:] = xT16[:, e * NSHARED:(e + 1) * NSHARED]
        scale = np.zeros((ttot, 1), dtype=np.float32)
        scale[:cnt, 0] = wt
        scale[cap:, 0] = 1.0
        in_maps.append({
            "xT": xin,
            "wg": np.asarray(w_gate[e]).astype(np.float16),
            "wu": np.asarray(w_up[e]).astype(np.float16),
            "wd": np.asarray(w_down[e]).astype(np.float16),
            "sg": sg16, "su": su16, "sd": sd16,
            "scale": scale,
        })

    res = run_bass_kernel_spmd(nc, in_maps, list(range(NCORES)), trace=trace)
    last_results = res

    out_full = np.zeros((N_TOKENS, D_MODEL), dtype=np.float32)
    for e in range(N_EXPERTS):
        r = res.results[e]["out"]
        cnt = len(idx_list[e])
        out_full[idx_list[e]] += r[:cnt]
        out_full[e * NSHARED:(e + 1) * NSHARED] += r[cap:]
    return out_full.reshape(B, T, D_MODEL)
